# revision 24
# baseline (speedup 1.0000x reference)
"""AUGRU cell (attention-scaled GRU update) on 8 Trainium2 NeuronCores.

Data-parallel: batch B=65536 sharded 8 ways (8192 rows/core); gate weights
replicated.  Per core (gate-major layout, batch on the free axis):

  gates_x = x @ W_x.T + b_x
  gates_h = h @ W_h.T + b_h
  u = sigmoid(U); r = sigmoid(R); t = tanh(Cx + r*Ch)
  h_new = h + att*u*(t - h)

v10 design — bf16 wire + paired epilogue + de-stalled ACT queue:
  - all wire tensors bf16: xT/hT/attb in, h_newT out (host up/down-casts);
    att pre-broadcast to [128, BL] on the host.
  - per group of 512 cols: 2 K=1 bias-prefill matmuls seed U|R so ONE
    merged sigmoid [128,1024] needs no bias operand; identity matmul
    accumulates m = (Ch+bCh)*r into the open Cx bank; tanh reads PSUM
    with bCx via the ACT bias operand.
  - tanh(g) is emitted one pipeline stage late so it never heads the ACT
    queue while waiting on the id-matmul (PSUM: pur bufs=2, pcx bufs=3,
    pch bufs=1 -> exactly 8 banks).
  - groups processed in PAIRS on the SBUF side: u/r and tanh outputs land
    in pair tiles [P, 2(gate/ph), ...] so u_att, q, ho run at FD=1024 on
    DVE (amortizes the fixed ~60-100cyc per-op cost); d = t - h runs
    per-group on GPSIMD; output DMA per pair from the sync queue.
"""

import sys

sys.path.insert(0, "/opt/trn_rl_repo")

import os
from contextlib import ExitStack

import numpy as np
import ml_dtypes

import concourse.bass as bass
import concourse.tile as tile
from concourse import bacc, mybir
from concourse.bass_utils import run_bass_kernel_spmd

F32 = mybir.dt.float32
BF16 = mybir.dt.bfloat16
F8 = mybir.dt.float8e4
F8NP = ml_dtypes.float8_e4m3fn
AF = mybir.ActivationFunctionType
OP = mybir.AluOpType
BFNP = ml_dtypes.bfloat16

B = 65536
NCORES = 8
BL = B // NCORES  # 8192 rows per core
I = 128
H = 128
P = 128
ROWS = 512  # batch rows per group (one fp32 PSUM bank per gate)
NGROUPS = BL // ROWS  # 16
CHG = 4  # groups per input DMA chunk
CH = CHG * ROWS  # 2048
NCHUNKS = NGROUPS // CHG  # 4
PR = 2 * ROWS  # pair width 1024


def build_program():
    nc = bacc.Bacc("TRN2", target_bir_lowering=False, debug=False)

    xT_d = nc.dram_tensor("xT", [I, BL], BF16, kind="ExternalInput").ap()
    hT_d = nc.dram_tensor("hT", [H, BL], BF16, kind="ExternalInput").ap()
    ab_d = nc.dram_tensor("attb", [P, BL], BF16, kind="ExternalInput").ap()
    wx_d = nc.dram_tensor("wxT", [I, 3, P], BF16, kind="ExternalInput").ap()
    wh_d = nc.dram_tensor("whT", [H, 3, P], BF16, kind="ExternalInput").ap()
    bc_d = nc.dram_tensor("bcol", [P, 4], F32, kind="ExternalInput").ap()
    id_d = nc.dram_tensor("ident", [P, P], BF16, kind="ExternalInput").ap()
    o_d = nc.dram_tensor("h_newT", [H, BL], BF16, kind="ExternalOutput").ap()

    with tile.TileContext(nc) as tc, ExitStack() as ctx:
        consts = ctx.enter_context(tc.tile_pool(name="consts", bufs=1))
        io = ctx.enter_context(tc.tile_pool(name="io", bufs=3))
        ep = ctx.enter_context(tc.tile_pool(name="ep", bufs=3))
        pp = ctx.enter_context(tc.tile_pool(name="pp", bufs=2))
        pur = ctx.enter_context(tc.tile_pool(name="pur", bufs=2, space="PSUM"))
        pcx = ctx.enter_context(tc.tile_pool(name="pcx", bufs=3, space="PSUM"))
        pch = ctx.enter_context(tc.tile_pool(name="pch", bufs=1, space="PSUM"))

        # ---------------- one-time setup ----------------
        wT = consts.tile([P, 6, P], BF16, tag="wT")  # [xu, xr, xc, hu, hr, hc]
        nc.sync.dma_start(wT[:, 0:3, :], wx_d)
        nc.sync.dma_start(wT[:, 3:6, :], wh_d)
        bcol = consts.tile([P, 4], F32, tag="bcol")  # [bU, bR, bCx, bCh]
        nc.sync.dma_start(bcol, bc_d)
        ident = consts.tile([P, P], BF16, tag="ident")
        nc.sync.dma_start(ident, id_d)

        xs = [None] * NCHUNKS
        hs = [None] * NCHUNKS
        ab = [None] * NCHUNKS
        stB = [None] * NGROUPS
        urp = [None] * (NGROUPS // 2)  # pair tiles: [P, 2(u/r), 2(ph), ROWS]
        tbp = [None] * (NGROUPS // 2)  # pair tiles: [P, 2(ph), ROWS]
        uap = [None] * (NGROUPS // 2)

        def stage_a(c):
            x = io.tile([P, CH], BF16, tag="xs")
            h = io.tile([P, CH], BF16, tag="hs")
            if c == 0:
                # split first chunk so the pipeline can start on the first half
                nc.sync.dma_start(x[:, 0:PR], xT_d[:, 0:PR])
                nc.sync.dma_start(h[:, 0:PR], hT_d[:, 0:PR])
                nc.sync.dma_start(x[:, PR:CH], xT_d[:, PR:CH])
                nc.sync.dma_start(h[:, PR:CH], hT_d[:, PR:CH])
            else:
                nc.sync.dma_start(x, xT_d[:, c * CH : (c + 1) * CH])
                nc.sync.dma_start(h, hT_d[:, c * CH : (c + 1) * CH])
            a = io.tile([P, CH], BF16, tag="ab")
            nc.sync.dma_start(a, ab_d[:, c * CH : (c + 1) * CH])
            xs[c], hs[c], ab[c] = x, h, a

        def stage_b(g):
            c, qi = g // CHG, g % CHG
            sl = slice(qi * ROWS, (qi + 1) * ROWS)
            xg, hg = xs[c][:, sl], hs[c][:, sl]
            gUR = pur.tile([P, 2, ROWS], F32, tag="gUR")
            gCx = pcx.tile([P, ROWS], F32, tag="gCx")
            gCh = pch.tile([P, ROWS], F32, tag="gCh")
            # bias prefill per bank (K=1 outer products); lhsT/rhs must sit on
            # partition 0, and a single N=1024 two-bank matmul is rejected
            nc.tensor.matmul(gUR[:, 0, :], lhsT=wT[:, 0, :], rhs=xg, start=True, stop=False)
            nc.tensor.matmul(gUR[:, 1, :], lhsT=wT[:, 1, :], rhs=xg, start=True, stop=False)
            nc.tensor.matmul(gUR[:, 0, :], lhsT=wT[:, 3, :], rhs=hg, start=False, stop=True)
            nc.tensor.matmul(gUR[:, 1, :], lhsT=wT[:, 4, :], rhs=hg, start=False, stop=True)
            nc.tensor.matmul(gCx, lhsT=wT[:, 2, :], rhs=xg, start=True, stop=False)  # stays open
            nc.tensor.matmul(gCh, lhsT=wT[:, 5, :], rhs=hg, start=True, stop=True)
            return gUR, gCx, gCh

        def stage_c(g):
            c, qi = g // CHG, g % CHG
            p, ph = g // 2, g % 2
            gUR, gCx, gCh = stB[g]
            if ph == 0:
                u = pp.tile([P, 2, 2, ROWS], BF16, tag="urp")
                urp[p] = u
            ur = urp[p]
            # sigmoid over [u|r] -> strided pair-tile slot ph
            nc.scalar.activation(ur[:, 0, ph, :], gUR[:, 0, :], AF.Sigmoid, bias=bcol[:, 0:1])
            nc.scalar.activation(ur[:, 1, ph, :], gUR[:, 1, :], AF.Sigmoid, bias=bcol[:, 1:2])
            m = ep.tile([P, ROWS], BF16, tag="m")
            nc.vector.scalar_tensor_tensor(
                m, in0=gCh, scalar=bcol[:, 3:4], in1=ur[:, 1, ph, :],
                op0=OP.add, op1=OP.mult,
            )
            nc.tensor.matmul(gCx, lhsT=ident, rhs=m, start=False, stop=True)
            if ph == 1:
                ua = pp.tile([P, PR], BF16, tag="uap")
                b0 = (g - 1) % CH // 1  # pair base within chunk
                pb = ((g - 1) % CHG) * ROWS
                nc.vector.tensor_tensor(ua, ur[:, 0, :, :], ab[c][:, pb : pb + PR], OP.mult)
                uap[p] = ua

        def stage_t(g):
            # tanh emitted one stage late: never heads the ACT queue stalled
            p, ph = g // 2, g % 2
            gUR, gCx, gCh = stB[g]
            if ph == 0:
                t = pp.tile([P, 2, ROWS], BF16, tag="tbp")
                tbp[p] = t
            nc.scalar.activation(tbp[p][:, ph, :], gCx, AF.Tanh, bias=bcol[:, 2:3])

        def stage_e(g):
            # runs at pair boundaries (g odd): d per group, q/ho/store per pair
            c = g // CHG
            p = g // 2
            g0 = g - 1
            pb = (g0 % CHG) * ROWS
            tb, ua = tbp[p], uap[p]
            d = ep.tile([P, PR], BF16, tag="d")
            if g == NGROUPS - 1:
                # last pair: split across engines + store per group to cut the tail
                nc.gpsimd.tensor_tensor(d[:, 0:ROWS], tb[:, 0, :], hs[c][:, pb : pb + ROWS], OP.subtract)
                nc.vector.tensor_tensor(d[:, ROWS:PR], tb[:, 1, :], hs[c][:, pb + ROWS : pb + PR], OP.subtract)
                q = ep.tile([P, PR], BF16, tag="q")
                ho = ep.tile([P, PR], BF16, tag="ho")
                for half in range(2):
                    hsl = slice(half * ROWS, (half + 1) * ROWS)
                    nc.vector.tensor_tensor(q[:, hsl], d[:, hsl], ua[:, hsl], OP.mult)
                    nc.vector.tensor_tensor(
                        ho[:, hsl], q[:, hsl], hs[c][:, pb + half * ROWS : pb + (half + 1) * ROWS], OP.add
                    )
                    nc.sync.dma_start(
                        o_d[:, (g0 + half) * ROWS : (g0 + half + 1) * ROWS], ho[:, hsl]
                    )
                return
            nc.gpsimd.tensor_tensor(d[:, 0:ROWS], tb[:, 0, :], hs[c][:, pb : pb + ROWS], OP.subtract)
            nc.gpsimd.tensor_tensor(d[:, ROWS:PR], tb[:, 1, :], hs[c][:, pb + ROWS : pb + PR], OP.subtract)
            q = ep.tile([P, PR], BF16, tag="q")
            nc.vector.tensor_tensor(q, d, ua, OP.mult)
            ho = ep.tile([P, PR], BF16, tag="ho")
            nc.vector.tensor_tensor(ho, q, hs[c][:, pb : pb + PR], OP.add)
            nc.sync.dma_start(o_d[:, g0 * ROWS : g0 * ROWS + PR], ho)

        for k in range(NGROUPS + 6):
            if k < NGROUPS and k % CHG == 0:
                stage_a(k // CHG)
            if 2 <= k < NGROUPS + 2:
                stB[k - 2] = stage_b(k - 2)
            if 3 <= k < NGROUPS + 3:
                stage_c(k - 3)
            if 4 <= k < NGROUPS + 4:
                stage_t(k - 4)
            if k >= 5 and (k - 5) % 2 == 1 and k - 5 < NGROUPS:
                stage_e(k - 5)

    nc.compile()
    return nc


_NC_CACHE = []


def _get_nc():
    if not _NC_CACHE:
        _NC_CACHE.append(build_program())
    return _NC_CACHE[0]


def make_in_maps(x, h_prev, att_score, W_x, b_x, W_h, b_h):
    """Shard + stage inputs for the 8 cores (bf16 wire format)."""
    x = np.asarray(x, dtype=np.float32)
    h_prev = np.asarray(h_prev, dtype=np.float32)
    att = np.asarray(att_score, dtype=np.float32)
    W_x = np.asarray(W_x, dtype=np.float32)
    W_h = np.asarray(W_h, dtype=np.float32)
    b_x = np.asarray(b_x, dtype=np.float32)
    b_h = np.asarray(b_h, dtype=np.float32)

    wxT = np.ascontiguousarray(W_x.T.reshape(I, 3, P).astype(BFNP))
    whT = np.ascontiguousarray(W_h.T.reshape(H, 3, P).astype(BFNP))
    bsum = b_x + b_h  # valid for U and R blocks
    bcol = np.stack(
        [bsum[0:P], bsum[P : 2 * P], b_x[2 * P : 3 * P], b_h[2 * P : 3 * P]], axis=1
    ).astype(np.float32)
    ident = np.eye(P, dtype=BFNP)

    in_maps = []
    for c in range(NCORES):
        s = slice(c * BL, (c + 1) * BL)
        attb = np.broadcast_to(att[s].astype(BFNP), (P, BL))
        in_maps.append(
            {
                "xT": np.ascontiguousarray(x[s].T.astype(BFNP)),
                "hT": np.ascontiguousarray(h_prev[s].T.astype(BFNP)),
                "attb": np.ascontiguousarray(attb),
                "wxT": wxT,
                "whT": whT,
                "bcol": bcol,
                "ident": ident,
            }
        )
    return in_maps


def kernel(x, h_prev, att_score, W_x, b_x, W_h, b_h, **_unused):
    nc = _get_nc()
    in_maps = make_in_maps(x, h_prev, att_score, W_x, b_x, W_h, b_h)
    res = run_bass_kernel_spmd(nc, in_maps, list(range(NCORES)))
    out = np.concatenate(
        [
            np.asarray(res.results[c]["h_newT"]).astype(np.float32).T
            for c in range(NCORES)
        ],
        axis=0,
    )
    return np.ascontiguousarray(out)


# revision 25
# speedup vs baseline: 1.0180x; 1.0180x over previous
"""AUGRU cell (attention-scaled GRU update) on 8 Trainium2 NeuronCores.

Data-parallel: batch B=65536 sharded 8 ways (8192 rows/core); gate weights
replicated.  Per core (gate-major layout, batch on the free axis):

  gates_x = x @ W_x.T + b_x
  gates_h = h @ W_h.T + b_h
  u = sigmoid(U); r = sigmoid(R); t = tanh(Cx + r*Ch)
  h_new = h + att*u*(t - h)

v10 design — bf16 wire + paired epilogue + de-stalled ACT queue:
  - all wire tensors bf16: xT/hT/attb in, h_newT out (host up/down-casts);
    att pre-broadcast to [128, BL] on the host.
  - per group of 512 cols: 2 K=1 bias-prefill matmuls seed U|R so ONE
    merged sigmoid [128,1024] needs no bias operand; identity matmul
    accumulates m = (Ch+bCh)*r into the open Cx bank; tanh reads PSUM
    with bCx via the ACT bias operand.
  - tanh(g) is emitted one pipeline stage late so it never heads the ACT
    queue while waiting on the id-matmul (PSUM: pur bufs=2, pcx bufs=3,
    pch bufs=1 -> exactly 8 banks).
  - groups processed in PAIRS on the SBUF side: u/r and tanh outputs land
    in pair tiles [P, 2(gate/ph), ...] so u_att, q, ho run at FD=1024 on
    DVE (amortizes the fixed ~60-100cyc per-op cost); d = t - h runs
    per-group on GPSIMD; output DMA per pair from the sync queue.
"""

import sys

sys.path.insert(0, "/opt/trn_rl_repo")

import os
from contextlib import ExitStack

import numpy as np
import ml_dtypes

import concourse.bass as bass
import concourse.tile as tile
from concourse import bacc, mybir
from concourse.bass_utils import run_bass_kernel_spmd

F32 = mybir.dt.float32
BF16 = mybir.dt.bfloat16
F8 = mybir.dt.float8e4
F8NP = ml_dtypes.float8_e4m3fn
AF = mybir.ActivationFunctionType
OP = mybir.AluOpType
BFNP = ml_dtypes.bfloat16

B = 65536
NCORES = 8
BL = B // NCORES  # 8192 rows per core
I = 128
H = 128
P = 128
ROWS = 512  # batch rows per group (one fp32 PSUM bank per gate)
NGROUPS = BL // ROWS  # 16
CHG = 4  # groups per input DMA chunk
CH = CHG * ROWS  # 2048
NCHUNKS = NGROUPS // CHG  # 4
PR = 2 * ROWS  # pair width 1024


def build_program():
    nc = bacc.Bacc("TRN2", target_bir_lowering=False, debug=False)

    xT_d = nc.dram_tensor("xT", [I, BL], BF16, kind="ExternalInput").ap()
    hT_d = nc.dram_tensor("hT", [H, BL], BF16, kind="ExternalInput").ap()
    ab_d = nc.dram_tensor("attb", [P, BL], BF16, kind="ExternalInput").ap()
    wx_d = nc.dram_tensor("wxT", [I, 3, P], BF16, kind="ExternalInput").ap()
    wh_d = nc.dram_tensor("whT", [H, 3, P], BF16, kind="ExternalInput").ap()
    bc_d = nc.dram_tensor("bcol", [P, 4], F32, kind="ExternalInput").ap()
    bu_d = nc.dram_tensor("bur2", [2, P], BF16, kind="ExternalInput").ap()
    se_d = nc.dram_tensor("sel", [2, 2 * ROWS], BF16, kind="ExternalInput").ap()
    id_d = nc.dram_tensor("ident", [P, P], BF16, kind="ExternalInput").ap()
    o_d = nc.dram_tensor("h_newT", [H, BL], BF16, kind="ExternalOutput").ap()

    with tile.TileContext(nc) as tc, ExitStack() as ctx:
        consts = ctx.enter_context(tc.tile_pool(name="consts", bufs=1))
        io = ctx.enter_context(tc.tile_pool(name="io", bufs=3))
        ep = ctx.enter_context(tc.tile_pool(name="ep", bufs=3))
        pp = ctx.enter_context(tc.tile_pool(name="pp", bufs=2))
        pur = ctx.enter_context(tc.tile_pool(name="pur", bufs=2, space="PSUM"))
        pcx = ctx.enter_context(tc.tile_pool(name="pcx", bufs=3, space="PSUM"))
        pch = ctx.enter_context(tc.tile_pool(name="pch", bufs=1, space="PSUM"))

        # ---------------- one-time setup ----------------
        wT = consts.tile([P, 6, P], BF16, tag="wT")  # [xu, xr, xc, hu, hr, hc]
        nc.sync.dma_start(wT[:, 0:3, :], wx_d)
        nc.sync.dma_start(wT[:, 3:6, :], wh_d)
        bcol = consts.tile([P, 4], F32, tag="bcol")  # [bU, bR, bCx, bCh]
        nc.sync.dma_start(bcol, bc_d)
        burT = consts.tile([1, 2 * P], BF16, tag="burT")  # [bU | bR] on partition 0
        nc.sync.dma_start(burT, bu_d.rearrange("a b -> (a b)").unsqueeze(0))
        sel = consts.tile([2, 2 * ROWS], BF16, tag="sel")  # row 0 = ones
        nc.sync.dma_start(sel, se_d)
        ident = consts.tile([P, P], BF16, tag="ident")
        nc.sync.dma_start(ident, id_d)

        xs = [None] * NCHUNKS
        hs = [None] * NCHUNKS
        ab = [None] * NCHUNKS
        stB = [None] * NGROUPS
        urp = [None] * (NGROUPS // 2)  # pair tiles: [P, 2(u/r), 2(ph), ROWS]
        tbp = [None] * (NGROUPS // 2)  # pair tiles: [P, 2(ph), ROWS]
        uap = [None] * (NGROUPS // 2)

        def stage_a(c):
            x = io.tile([P, CH], BF16, tag="xs")
            h = io.tile([P, CH], BF16, tag="hs")
            if c == 0:
                # split first chunk so the pipeline can start on the first half
                nc.sync.dma_start(x[:, 0:PR], xT_d[:, 0:PR])
                nc.sync.dma_start(h[:, 0:PR], hT_d[:, 0:PR])
                nc.sync.dma_start(x[:, PR:CH], xT_d[:, PR:CH])
                nc.sync.dma_start(h[:, PR:CH], hT_d[:, PR:CH])
            else:
                nc.sync.dma_start(x, xT_d[:, c * CH : (c + 1) * CH])
                nc.sync.dma_start(h, hT_d[:, c * CH : (c + 1) * CH])
            a = io.tile([P, CH], BF16, tag="ab")
            nc.sync.dma_start(a, ab_d[:, c * CH : (c + 1) * CH])
            xs[c], hs[c], ab[c] = x, h, a

        def stage_b(g):
            c, qi = g // CHG, g % CHG
            sl = slice(qi * ROWS, (qi + 1) * ROWS)
            xg, hg = xs[c][:, sl], hs[c][:, sl]
            gUR = pur.tile([P, 2, ROWS], F32, tag="gUR")
            gCx = pcx.tile([P, ROWS], F32, tag="gCx")
            gCh = pch.tile([P, ROWS], F32, tag="gCh")
            # bias prefill per bank (K=1 outer products); lhsT/rhs must sit on
            # partition 0, and a single N=1024 two-bank matmul is rejected
            nc.tensor.matmul(gUR[:, 0, :], lhsT=burT[:, 0:P], rhs=sel[0:1, 0:ROWS], start=True, stop=False)
            nc.tensor.matmul(gUR[:, 1, :], lhsT=burT[:, P : 2 * P], rhs=sel[0:1, 0:ROWS], start=True, stop=False)
            nc.tensor.matmul(gUR[:, 0, :], lhsT=wT[:, 0, :], rhs=xg, start=False, stop=False)
            nc.tensor.matmul(gUR[:, 1, :], lhsT=wT[:, 1, :], rhs=xg, start=False, stop=False)
            nc.tensor.matmul(gUR[:, 0, :], lhsT=wT[:, 3, :], rhs=hg, start=False, stop=True)
            nc.tensor.matmul(gUR[:, 1, :], lhsT=wT[:, 4, :], rhs=hg, start=False, stop=True)
            nc.tensor.matmul(gCx, lhsT=wT[:, 2, :], rhs=xg, start=True, stop=False)  # stays open
            nc.tensor.matmul(gCh, lhsT=wT[:, 5, :], rhs=hg, start=True, stop=True)
            return gUR, gCx, gCh

        def stage_c(g):
            c, qi = g // CHG, g % CHG
            p, ph = g // 2, g % 2
            gUR, gCx, gCh = stB[g]
            if ph == 0:
                u = pp.tile([P, 2, 2, ROWS], BF16, tag="urp")
                urp[p] = u
            ur = urp[p]
            # sigmoid over [u|r] -> strided pair-tile slot ph
            nc.scalar.activation(ur[:, :, ph, :], gUR, AF.Sigmoid)
            m = ep.tile([P, ROWS], BF16, tag="m")
            nc.vector.scalar_tensor_tensor(
                m, in0=gCh, scalar=bcol[:, 3:4], in1=ur[:, 1, ph, :],
                op0=OP.add, op1=OP.mult,
            )
            nc.tensor.matmul(gCx, lhsT=ident, rhs=m, start=False, stop=True)
            if ph == 1:
                ua = pp.tile([P, PR], BF16, tag="uap")
                b0 = (g - 1) % CH // 1  # pair base within chunk
                pb = ((g - 1) % CHG) * ROWS
                nc.vector.tensor_tensor(ua, ur[:, 0, :, :], ab[c][:, pb : pb + PR], OP.mult)
                uap[p] = ua

        def stage_t(g):
            # tanh emitted one stage late: never heads the ACT queue stalled
            p, ph = g // 2, g % 2
            gUR, gCx, gCh = stB[g]
            if ph == 0:
                t = pp.tile([P, 2, ROWS], BF16, tag="tbp")
                tbp[p] = t
            nc.scalar.activation(tbp[p][:, ph, :], gCx, AF.Tanh, bias=bcol[:, 2:3])

        def stage_e(g):
            # runs at pair boundaries (g odd): d per group, q/ho/store per pair
            c = g // CHG
            p = g // 2
            g0 = g - 1
            pb = (g0 % CHG) * ROWS
            tb, ua = tbp[p], uap[p]
            d = ep.tile([P, PR], BF16, tag="d")
            if g == NGROUPS - 1:
                # last pair: split across engines + store per group to cut the tail
                nc.gpsimd.tensor_tensor(d[:, 0:ROWS], tb[:, 0, :], hs[c][:, pb : pb + ROWS], OP.subtract)
                nc.vector.tensor_tensor(d[:, ROWS:PR], tb[:, 1, :], hs[c][:, pb + ROWS : pb + PR], OP.subtract)
                q = ep.tile([P, PR], BF16, tag="q")
                ho = ep.tile([P, PR], BF16, tag="ho")
                for half in range(2):
                    hsl = slice(half * ROWS, (half + 1) * ROWS)
                    nc.vector.tensor_tensor(q[:, hsl], d[:, hsl], ua[:, hsl], OP.mult)
                    nc.vector.tensor_tensor(
                        ho[:, hsl], q[:, hsl], hs[c][:, pb + half * ROWS : pb + (half + 1) * ROWS], OP.add
                    )
                    nc.sync.dma_start(
                        o_d[:, (g0 + half) * ROWS : (g0 + half + 1) * ROWS], ho[:, hsl]
                    )
                return
            nc.gpsimd.tensor_tensor(d[:, 0:ROWS], tb[:, 0, :], hs[c][:, pb : pb + ROWS], OP.subtract)
            nc.gpsimd.tensor_tensor(d[:, ROWS:PR], tb[:, 1, :], hs[c][:, pb + ROWS : pb + PR], OP.subtract)
            q = ep.tile([P, PR], BF16, tag="q")
            nc.vector.tensor_tensor(q, d, ua, OP.mult)
            ho = ep.tile([P, PR], BF16, tag="ho")
            nc.vector.tensor_tensor(ho, q, hs[c][:, pb : pb + PR], OP.add)
            nc.sync.dma_start(o_d[:, g0 * ROWS : g0 * ROWS + PR], ho)

        for k in range(NGROUPS + 6):
            if k < NGROUPS and k % CHG == 0:
                stage_a(k // CHG)
            if 2 <= k < NGROUPS + 2:
                stB[k - 2] = stage_b(k - 2)
            if 3 <= k < NGROUPS + 3:
                stage_c(k - 3)
            if 4 <= k < NGROUPS + 4:
                stage_t(k - 4)
            if k >= 5 and (k - 5) % 2 == 1 and k - 5 < NGROUPS:
                stage_e(k - 5)

    nc.compile()
    return nc


_NC_CACHE = []


def _get_nc():
    if not _NC_CACHE:
        _NC_CACHE.append(build_program())
    return _NC_CACHE[0]


def make_in_maps(x, h_prev, att_score, W_x, b_x, W_h, b_h):
    """Shard + stage inputs for the 8 cores (bf16 wire format)."""
    x = np.asarray(x, dtype=np.float32)
    h_prev = np.asarray(h_prev, dtype=np.float32)
    att = np.asarray(att_score, dtype=np.float32)
    W_x = np.asarray(W_x, dtype=np.float32)
    W_h = np.asarray(W_h, dtype=np.float32)
    b_x = np.asarray(b_x, dtype=np.float32)
    b_h = np.asarray(b_h, dtype=np.float32)

    wxT = np.ascontiguousarray(W_x.T.reshape(I, 3, P).astype(BFNP))
    whT = np.ascontiguousarray(W_h.T.reshape(H, 3, P).astype(BFNP))
    bsum = b_x + b_h  # valid for U and R blocks
    bcol = np.stack(
        [bsum[0:P], bsum[P : 2 * P], b_x[2 * P : 3 * P], b_h[2 * P : 3 * P]], axis=1
    ).astype(np.float32)
    bur2 = np.ascontiguousarray(bsum[0 : 2 * P].reshape(2, P).astype(BFNP))
    sel = np.zeros((2, 2 * ROWS), dtype=BFNP)
    sel[0, 0:ROWS] = 1
    sel[1, ROWS : 2 * ROWS] = 1
    ident = np.eye(P, dtype=BFNP)

    in_maps = []
    for c in range(NCORES):
        s = slice(c * BL, (c + 1) * BL)
        attb = np.broadcast_to(att[s].astype(BFNP), (P, BL))
        in_maps.append(
            {
                "xT": np.ascontiguousarray(x[s].T.astype(BFNP)),
                "hT": np.ascontiguousarray(h_prev[s].T.astype(BFNP)),
                "attb": np.ascontiguousarray(attb),
                "wxT": wxT,
                "whT": whT,
                "bcol": bcol,
                "bur2": bur2,
                "sel": sel,
                "ident": ident,
            }
        )
    return in_maps


def kernel(x, h_prev, att_score, W_x, b_x, W_h, b_h, **_unused):
    nc = _get_nc()
    in_maps = make_in_maps(x, h_prev, att_score, W_x, b_x, W_h, b_h)
    res = run_bass_kernel_spmd(nc, in_maps, list(range(NCORES)))
    out = np.concatenate(
        [
            np.asarray(res.results[c]["h_newT"]).astype(np.float32).T
            for c in range(NCORES)
        ],
        axis=0,
    )
    return np.ascontiguousarray(out)
